# revision 1
# baseline (speedup 1.0000x reference)
"""ClusterInversionLoss Trainium2 kernel.

Strategy (data-parallel over the flat pair list, per sharding hint):
  - Host: co-locate each pair's data by gathering rows at pair_i/pair_j
    (l0-shifted logits per class, target delta, sample weights), shard
    the 2M pairs evenly across 8 cores, per-core planes (128, 11, L) bf16.
  - Device (per core, pure streaming, no random access):
      s = softmax-expected-score for both pair sides: exp on ACT (one
      table set, pinned), Z and W = sum c*e_c via a shared suffix-sum
      add chain in bf16 on DVE, 1/Z = exp(-ln Z) on ACT, pair chain on
      the Pool engine, softplus via exp/ln with the free affine bias,
      fused multiply + per-partition reduce via scalar_tensor_tensor.
  - Host: sum the 8x128 partial (loss, weight) pairs, return the ratio.

Computes exactly the reference quantity; only rows referenced by pairs
contribute to the loss, so unpaired rows need not be touched.
"""

import numpy as np

import concourse.bacc as bacc
import concourse.mybir as mybir
from concourse.bass_utils import run_bass_kernel_spmd
from concourse.tile import TileContext

NCORES = 8
NPAIRS = 2_000_000
PC = NPAIRS // NCORES  # 250_000 pairs per core
P = 128
NCHUNK = 2
LC = 978
L = NCHUNK * LC  # 1956 columns per partition; P*L = 250_368 >= PC
NPLANE = 11  # li1'..li4', lj1'..lj4' (l0-shifted logits), dy, wi, wj

EPS = 1e-8

f32 = mybir.dt.float32
bf16 = mybir.dt.bfloat16
AF = mybir.ActivationFunctionType
ALU = mybir.AluOpType


def _pin_act_tables(arch):
    """Make every ACT function we use first-match to one table set that
    contains both exp and ln, so the kernel needs a single
    ACT_TABLE_LOAD instead of thrashing between the exp-only and
    ln-only sets (1.3us per reload).  Only membership of the cached
    selection dict is edited; set indices (act_func_set_id) and the
    real on-device tables are untouched, so lowering stays correct.
    """
    from concourse.hw_specs import get_activation_tables

    tabs = get_activation_tables(arch)
    ours = {AF.Exp, AF.Ln, AF.Sign, AF.Abs, AF.Square}
    combined = None
    for name, fns in tabs.items():
        if ours <= fns:
            combined = name
            break
    if combined is None:
        return
    for name, fns in tabs.items():
        if name != combined:
            fns -= ours


def _build():
    nc = bacc.Bacc("TRN2", target_bir_lowering=False)
    _pin_act_tables(nc.m.arch)
    X = nc.dram_tensor("x", [P, NPLANE, L], bf16, kind="ExternalInput")
    OUT = nc.dram_tensor("out", [P, 2], f32, kind="ExternalOutput")

    with TileContext(nc) as tc:
        with (
            tc.tile_pool(name="io", bufs=2) as io,
            tc.tile_pool(name="ew", bufs=2) as ew,
            tc.tile_pool(name="sc", bufs=2) as sc,
            tc.tile_pool(name="s1", bufs=1) as s1p,
            tc.tile_pool(name="acc", bufs=1) as accp,
        ):
            accL = [accp.tile([P, 1], f32, tag=f"accL{c}", name=f"accL{c}")
                    for c in range(NCHUNK)]
            accW = [accp.tile([P, 1], f32, tag=f"accW{c}", name=f"accW{c}")
                    for c in range(NCHUNK)]
            ST = [{} for _ in range(NCHUNK)]

            def stage1(c):
                """DMA, exp, suffix-sum Z/W, reciprocal-of-Z."""
                t = ST[c]
                cs = slice(c * LC, (c + 1) * LC)
                LI = io.tile([P, 4, LC], bf16, tag="LI", name="LI")
                nc.sync.dma_start(out=LI[:], in_=X[:, 0:4, cs])
                LJ = io.tile([P, 4, LC], bf16, tag="LJ", name="LJ")
                nc.sync.dma_start(out=LJ[:], in_=X[:, 4:8, cs])
                DY = io.tile([P, LC], bf16, tag="DY", name="DY")
                nc.sync.dma_start(out=DY[:], in_=X[:, 8, cs])
                WI = io.tile([P, LC], bf16, tag="WI", name="WI")
                nc.sync.dma_start(out=WI[:], in_=X[:, 9, cs])
                WJ = io.tile([P, LC], bf16, tag="WJ", name="WJ")
                nc.sync.dma_start(out=WJ[:], in_=X[:, 10, cs])
                t.update(DY=DY, WI=WI, WJ=WJ)

                EI = ew.tile([P, 4, LC], bf16, tag="EI", name="EI")
                nc.scalar.activation(EI[:], LI[:], AF.Exp)
                EJ = ew.tile([P, 4, LC], bf16, tag="EJ", name="EJ")
                nc.scalar.activation(EJ[:], LJ[:], AF.Exp)

                # suffix-sum chains: A=e3+e4; B=e2+A; T1=e1+B; Z=1+T1;
                # U=T1+B; V=A+e4; W=U+V = e1+2e2+3e3+4e4
                ZIJ = sc.tile([P, 2, LC], bf16, tag="ZIJ", name="ZIJ")
                WT = {}
                for side, (E, an, bn, tn) in enumerate(
                        ((EI, "Ai", "Bi", "Ti"), (EJ, "Aj", "Bj", "Tj"))):
                    A = sc.tile([P, LC], bf16, tag=an, name=an)
                    B = sc.tile([P, LC], bf16, tag=bn, name=bn)
                    T = sc.tile([P, LC], bf16, tag=tn, name=tn)
                    nc.vector.tensor_add(out=A[:], in0=E[:, 2, :], in1=E[:, 3, :])
                    nc.vector.tensor_add(out=B[:], in0=E[:, 1, :], in1=A[:])
                    nc.vector.tensor_add(out=T[:], in0=E[:, 0, :], in1=B[:])
                    nc.vector.tensor_scalar_add(out=ZIJ[:, side, :], in0=T[:],
                                                scalar1=1.0)
                    nc.vector.tensor_add(out=B[:], in0=T[:], in1=B[:])
                    nc.vector.tensor_add(out=A[:], in0=A[:], in1=E[:, 3, :])
                    nc.vector.tensor_add(out=T[:], in0=B[:], in1=A[:])
                    WT[side] = T
                t["WT"] = WT

                ZT = s1p.tile([P, 2, LC], f32, tag="ZT", name="ZT")
                nc.scalar.activation(ZT[:], ZIJ[:], AF.Ln)
                RZ = s1p.tile([P, 2, LC], f32, tag="RZ", name="RZ")
                nc.scalar.activation(RZ[:], ZT[:], AF.Exp, scale=-1.0)
                t["RZ"] = RZ

            def stage2(c):
                """sign/abs, pair chain on Pool, softplus, fused reduces."""
                t = ST[c]
                DY, WI, WJ, RZ, WT = t["DY"], t["WI"], t["WJ"], t["RZ"], t["WT"]
                SG = s1p.tile([P, LC], f32, tag="SG", name="SG")
                nc.scalar.activation(SG[:], DY[:], AF.Sign)
                DIST = s1p.tile([P, LC], f32, tag="DIST", name="DIST")
                nc.scalar.activation(DIST[:], DY[:], AF.Abs)

                Si = s1p.tile([P, LC], f32, tag="Si", name="Si")
                Sj = s1p.tile([P, LC], f32, tag="Sj", name="Sj")
                nc.gpsimd.tensor_mul(out=Si[:], in0=WT[0][:], in1=RZ[:, 0, :])
                nc.gpsimd.tensor_mul(out=Sj[:], in0=WT[1][:], in1=RZ[:, 1, :])
                nc.gpsimd.tensor_sub(out=Si[:], in0=Si[:], in1=Sj[:])
                S1 = s1p.tile([P, LC], f32, tag="S1", name="S1")
                nc.gpsimd.tensor_mul(out=S1[:], in0=SG[:], in1=Si[:])
                WS = s1p.tile([P, LC], bf16, tag="WS", name="WS")
                nc.gpsimd.tensor_add(out=WS[:], in0=WI[:], in1=WJ[:])
                S2C = s1p.tile([P, LC], f32, tag="S2C", name="S2C")
                nc.gpsimd.tensor_mul(out=S2C[:], in0=DIST[:], in1=WS[:])

                S2 = s1p.tile([P, LC], f32, tag="S2", name="S2")
                nc.scalar.activation(S2[:], S1[:], AF.Exp, scale=-1.0)
                nc.scalar.activation(S1[:], S2[:], AF.Ln, bias=1.0)
                S3 = s1p.tile([P, LC], f32, tag="S3", name="S3")
                nc.vector.tensor_scalar(
                    out=S3[:], in0=DIST[:], scalar1=1.0, scalar2=None, op0=ALU.min)

                S5 = s1p.tile([P, LC], f32, tag="S5", name="S5")
                nc.vector.scalar_tensor_tensor(
                    out=S5[:], in0=S1[:], scalar=1.0, in1=S2C[:],
                    op0=ALU.mult, op1=ALU.mult, accum_out=accL[c][:])
                nc.vector.scalar_tensor_tensor(
                    out=S1[:], in0=S3[:], scalar=1.0, in1=WS[:],
                    op0=ALU.mult, op1=ALU.mult, accum_out=accW[c][:])
                if c > 0:
                    nc.vector.tensor_add(out=accL[c][:], in0=accL[c][:],
                                         in1=accL[c - 1][:])
                    nc.vector.tensor_add(out=accW[c][:], in0=accW[c][:],
                                         in1=accW[c - 1][:])

            # staggered emission: engines run their streams in order, so
            # chunk c+1's front-half is emitted before chunk c's tail to
            # let the phases pipeline across engines.
            for c in range(NCHUNK):
                stage1(c)
                stage2(c)

            nc.sync.dma_start(out=OUT[:, 0:1], in_=accL[NCHUNK - 1][:])
            nc.sync.dma_start(out=OUT[:, 1:2], in_=accW[NCHUNK - 1][:])

    nc.compile()
    return nc


_NC_CACHE = {}


def _get_nc():
    if "nc" not in _NC_CACHE:
        _NC_CACHE["nc"] = _build()
    return _NC_CACHE["nc"]


def _prepare(inputs, targets, cluster_ids, sample_weight, pair_i, pair_j):
    import ml_dtypes

    x = np.ascontiguousarray(np.asarray(inputs), dtype=np.float32)
    t = np.asarray(targets)
    w = np.asarray(sample_weight, dtype=np.float32)
    pi = np.asarray(pair_i).astype(np.int64, copy=False)
    pj = np.asarray(pair_j).astype(np.int64, copy=False)

    li = x[pi]  # (NPAIRS, 5)
    lj = x[pj]
    lis = li[:, 1:5] - li[:, 0:1]  # l0-shift: softmax is shift-invariant
    ljs = lj[:, 1:5] - lj[:, 0:1]
    dy = (t[pi] - t[pj]).astype(np.float32)
    wi = w[pi]
    wj = w[pj]

    PL = P * L
    bf = ml_dtypes.bfloat16
    maps = []
    for k in range(NCORES):
        sl = slice(k * PC, (k + 1) * PC)
        A = np.zeros((P, NPLANE, L), dtype=bf)

        def put(plane, v):
            vv = np.zeros(PL, dtype=np.float32)
            vv[:PC] = v
            A[:, plane, :] = vv.reshape(P, L).astype(bf)

        for ccls in range(4):
            put(ccls, lis[sl][:, ccls])
            put(4 + ccls, ljs[sl][:, ccls])
        put(8, dy[sl])
        put(9, wi[sl])
        put(10, wj[sl])
        maps.append({"x": A})
    return maps


def _run(in_maps, trace=False, **kw):
    nc = _get_nc()
    return run_bass_kernel_spmd(nc, in_maps, list(range(NCORES)), trace=trace, **kw)


def kernel(inputs, targets, cluster_ids, sample_weight, pair_i, pair_j):
    in_maps = _prepare(inputs, targets, cluster_ids, sample_weight, pair_i, pair_j)
    res = _run(in_maps)
    tl = 0.0
    tw = 0.0
    for k in range(NCORES):
        o = res.results[k]["out"]
        tl += float(o[:, 0].sum(dtype=np.float64))
        tw += float(o[:, 1].sum(dtype=np.float64))
    # the 0.5 pair-weight factor cancels in the ratio; fold it into eps
    return np.float32(tl / (tw + 2 * EPS))



# revision 4
# speedup vs baseline: 1.5644x; 1.5644x over previous
"""ClusterInversionLoss Trainium2 kernel.

Strategy (data-parallel over the flat pair list, per sharding hint):
  - Host: gather each pair's rows, orient every pair so sign=+1 (swap
    i/j when y_i<y_j; ties contribute 0 via wd=0), l0-shift the logits
    (softmax shift invariance), fold |dy|*w_pair into a single wd plane,
    and pack per core a (128, 16384) bf16 matrix whose partition dim
    interleaves 31 pair-groups x 4 shifted logits (+ a constant
    zero-logit row that exp turns into the softmax "+1"), with the i/j
    sides of a pair in adjacent columns.  total_weight is a pure
    function of the inputs (no softmax), summed on host.
  - Device (per core): exp on ACT; Z=1+sum(e) and W=sum(c*e) via
    128x32-column-tiled matmuls on the otherwise-idle Tensor engine;
    1/Z via the single-instruction DVE reciprocal_approx_fast;
    s=W*(1/Z) and delta=s_i-s_j on DVE; softplus(-delta)=ln(1+exp(-d))
    on ACT (exp+ln share one table set); fused multiply-by-wd +
    per-partition reduce on DVE, chained across rounds via the reduce
    initial-value operand.
  - Host: sum the 8x128 loss partials, divide by host total_weight.

Computes exactly the reference quantity; only rows referenced by pairs
contribute, so unpaired rows need not be touched.
"""

import numpy as np

import concourse.bacc as bacc
import concourse.mybir as mybir
from concourse.bass_utils import run_bass_kernel_spmd
from concourse.tile import TileContext

NCORES = 8
NPAIRS = 2_000_000
PC = NPAIRS // NCORES   # 250_000 pairs per core
P = 128

G = 31                  # pair-groups per column (partition = 4*g + c)
ONES_ROW = 124          # constant zero-logit row -> exp() == 1 (the +1 in Z)
F = 16_384              # x columns per core
PC_PAD = (F // 2) * G   # 253_952 padded pairs per core >= PC
SRC = 4_096             # x columns per super-round
NSR = F // SRC          # 4 super-rounds
NJ = 2                  # PSUM sub-chunks per super-round (2048 cols)
NK = 4                  # matmul partition-blocks per sub-chunk
MB = 512                # matmul moving free dim (one PSUM bank)
TD = MB // 2            # delta columns per (j, k) block

EPS = 1e-8

f32 = mybir.dt.float32
bf16 = mybir.dt.bfloat16
AF = mybir.ActivationFunctionType
ALU = mybir.AluOpType


def _pin_act_tables(arch):
    """Make every ACT function we use first-match to one table set that
    contains both exp and ln, so the kernel needs a single
    ACT_TABLE_LOAD instead of thrashing between the exp-only and
    ln-only sets (1.3us per reload).  Only membership of the cached
    selection dict is edited; set indices (act_func_set_id) and the
    real on-device tables are untouched, so lowering stays correct.
    """
    from concourse.hw_specs import get_activation_tables

    tabs = get_activation_tables(arch)
    ours = {AF.Exp, AF.Ln}
    combined = None
    for name, fns in tabs.items():
        if ours <= fns:
            combined = name
            break
    if combined is None:
        return
    for name, fns in tabs.items():
        if name != combined:
            fns -= ours


def _build():
    nc = bacc.Bacc("TRN2", target_bir_lowering=False)
    _pin_act_tables(nc.m.arch)
    X = nc.dram_tensor("x", [P, F], bf16, kind="ExternalInput")
    WD = nc.dram_tensor("wd", [P, NSR, NJ, TD], bf16, kind="ExternalInput")
    WZT = nc.dram_tensor("wzt", [P, 32], bf16, kind="ExternalInput")
    WWT = nc.dram_tensor("wwt", [P, 32], bf16, kind="ExternalInput")
    OUT = nc.dram_tensor("out", [P, 1], f32, kind="ExternalOutput")

    with TileContext(nc) as tc:
        with (
            tc.tile_pool(name="io", bufs=2) as io,
            tc.tile_pool(name="ew", bufs=2) as ew,
            tc.tile_pool(name="ps", bufs=2, space="PSUM") as ps,
            tc.tile_pool(name="s1", bufs=2) as s1,
            tc.tile_pool(name="cst", bufs=1) as cst,
            tc.tile_pool(name="acc", bufs=1) as accp,
        ):
            wz = cst.tile([P, 32], bf16, tag="wz", name="wz")
            nc.sync.dma_start(out=wz[:], in_=WZT[:, :])
            ww = cst.tile([P, 32], bf16, tag="ww", name="ww")
            nc.sync.dma_start(out=ww[:], in_=WWT[:, :])
            wdt = cst.tile([P, NSR, NJ, TD], bf16, tag="wdt", name="wdt")
            nc.sync.dma_start(out=wdt[:], in_=WD[:, :, :, :])

            accs = [accp.tile([P, 1], f32, tag=f"acc{i}", name=f"acc{i}")
                    for i in range(NSR)]

            def super_round(sr):
                xt = io.tile([P, SRC], bf16, tag="x", name=f"x{sr}")
                nc.sync.dma_start(out=xt[:], in_=X[:, sr * SRC:(sr + 1) * SRC])
                E = ew.tile([P, SRC], bf16, tag="E", name=f"E{sr}")
                nc.scalar.activation(E[:], xt[:], AF.Exp)

                Zt = ps.tile([P, NJ, MB], f32, tag="Z", name=f"Z{sr}")
                Wt = ps.tile([P, NJ, MB], f32, tag="W", name=f"W{sr}")
                for j in range(NJ):
                    for k in range(NK):
                        rhs = E[:, j * (NK * MB) + k * MB:
                                j * (NK * MB) + (k + 1) * MB]
                        nc.tensor.matmul(
                            Zt[32 * k:32 * (k + 1), j], wz[:, :], rhs,
                            start=True, stop=True, tile_position=(0, 32 * k))
                    for k in range(NK):
                        rhs = E[:, j * (NK * MB) + k * MB:
                                j * (NK * MB) + (k + 1) * MB]
                        nc.tensor.matmul(
                            Wt[32 * k:32 * (k + 1), j], ww[:, :], rhs,
                            start=True, stop=True, tile_position=(0, 32 * k))

                RZ = s1.tile([P, NJ, MB], f32, tag="RZ", name=f"RZ{sr}")
                nc.vector.reciprocal_approx_fast(out=RZ[:], in_=Zt[:])
                S = s1.tile([P, NJ, TD, 2], bf16, tag="S", name=f"S{sr}")
                nc.vector.tensor_mul(out=S[:], in0=Wt[:], in1=RZ[:])
                D = s1.tile([P, NJ, TD], bf16, tag="D", name=f"D{sr}")
                nc.vector.tensor_sub(out=D[:], in0=S[:, :, :, 0],
                                     in1=S[:, :, :, 1])
                U = s1.tile([P, NJ, TD], bf16, tag="U", name=f"U{sr}")
                nc.scalar.activation(U[:], D[:], AF.Exp, scale=-1.0)
                SP = s1.tile([P, NJ, TD], bf16, tag="SP", name=f"SP{sr}")
                nc.scalar.activation(SP[:], U[:], AF.Ln, bias=1.0)
                SC = s1.tile([P, NJ, TD], bf16, tag="SC", name=f"SC{sr}")
                nc.vector.scalar_tensor_tensor(
                    out=SC[:], in0=SP[:], scalar=1.0, in1=wdt[:, sr],
                    op0=ALU.mult, op1=ALU.mult, accum_out=accs[sr][:])
                if sr > 0:
                    nc.vector.tensor_add(out=accs[sr][:], in0=accs[sr][:],
                                         in1=accs[sr - 1][:])

            for sr in range(NSR):
                super_round(sr)

            nc.sync.dma_start(out=OUT[:, :], in_=accs[NSR - 1][:])

    nc.compile()
    return nc


_NC_CACHE = {}


def _get_nc():
    if "nc" not in _NC_CACHE:
        _NC_CACHE["nc"] = _build()
    return _NC_CACHE["nc"]


def _weights():
    # lhsT [K=128, M=32]: column g (< G) sums the 4 class-exps of group g;
    # WZT also picks up the constant-1 row (softmax +1).  Column 31 is a
    # padding output fed by all rows so its Z/W stay wholesome (no 1/0 in
    # reciprocal); its wd is always 0 so it never contributes.
    wzt = np.zeros((P, 32), np.float32)
    wwt = np.zeros((P, 32), np.float32)
    for g in range(G):
        for c in range(4):
            wzt[4 * g + c, g] = 1.0
            wwt[4 * g + c, g] = float(c + 1)
    wzt[ONES_ROW, :G] = 1.0
    wzt[:, 31] = 1.0
    wwt[:, 31] = 1.0
    return wzt, wwt


def _prepare(inputs, targets, cluster_ids, sample_weight, pair_i, pair_j):
    import ml_dtypes

    bf = ml_dtypes.bfloat16
    x = np.ascontiguousarray(np.asarray(inputs), dtype=np.float32)
    t = np.asarray(targets)
    w = np.asarray(sample_weight, dtype=np.float32)
    pi = np.asarray(pair_i).astype(np.int64, copy=False)
    pj = np.asarray(pair_j).astype(np.int64, copy=False)

    dy = (t[pi] - t[pj]).astype(np.int64)
    swap = dy < 0
    pi2 = np.where(swap, pj, pi)
    pj2 = np.where(swap, pi, pj)
    dist = np.abs(dy).astype(np.float32)

    li = x[pi2]                         # (NPAIRS, 5), oriented so s_i-s_j
    lj = x[pj2]
    lsi = li[:, 1:5] - li[:, 0:1]       # l0-shift: softmax shift-invariant
    lsj = lj[:, 1:5] - lj[:, 0:1]

    wp = 0.5 * (w[pi] + w[pj])          # symmetric under swap
    wd = dist * wp                      # 0 exactly for ties (inactive)
    twa = float((wp * (dist != 0)).sum(dtype=np.float64))

    wzt, wwt = _weights()
    wzt = wzt.astype(bf)
    wwt = wwt.astype(bf)

    B = F // 2
    maps = []
    for kcore in range(NCORES):
        sl = slice(kcore * PC, (kcore + 1) * PC)

        lsi_p = np.zeros((PC_PAD, 4), np.float32)
        lsi_p[:PC] = lsi[sl]
        lsj_p = np.zeros((PC_PAD, 4), np.float32)
        lsj_p[:PC] = lsj[sl]
        wd_p = np.zeros(PC_PAD, np.float32)
        wd_p[:PC] = wd[sl]

        # x_dev[4g+c, 2b+side] = logit c of side of pair q = G*b+g
        lsi_r = lsi_p.reshape(B, G, 4)          # [b, g, c]
        lsj_r = lsj_p.reshape(B, G, 4)
        x4 = np.stack([lsi_r, lsj_r], axis=3)   # [b, g, c, side]
        x_dev = np.zeros((P, F), np.float32)
        x_dev[:4 * G] = x4.transpose(1, 2, 0, 3).reshape(4 * G, F)
        x_dev = np.ascontiguousarray(x_dev).astype(bf)

        # wd_dev[32k+g, sr, j, t] = wd[q], q = G*(((sr*NJ+j)*NK+k)*TD+t)+g
        wd_r = wd_p.reshape(NSR, NJ, NK, TD, G)  # [sr, j, k, t, g]
        wd_r = wd_r.transpose(2, 4, 0, 1, 3)     # [k, g, sr, j, t]
        wd_dev = np.zeros((NK, 32, NSR, NJ, TD), np.float32)
        wd_dev[:, :G] = wd_r
        wd_dev = np.ascontiguousarray(
            wd_dev.reshape(P, NSR, NJ, TD)).astype(bf)

        maps.append({"x": x_dev, "wd": wd_dev, "wzt": wzt, "wwt": wwt})
    return maps, twa


def _run(in_maps, trace=False, **kw):
    nc = _get_nc()
    return run_bass_kernel_spmd(nc, in_maps, list(range(NCORES)), trace=trace, **kw)


def kernel(inputs, targets, cluster_ids, sample_weight, pair_i, pair_j):
    in_maps, twa = _prepare(inputs, targets, cluster_ids, sample_weight,
                            pair_i, pair_j)
    res = _run(in_maps)
    tl = 0.0
    for k in range(NCORES):
        o = res.results[k]["out"]
        tl += float(o[:, 0].sum(dtype=np.float64))
    return np.float32(tl / (twa + EPS))


# revision 9
# speedup vs baseline: 1.6852x; 1.0773x over previous
"""ClusterInversionLoss Trainium2 kernel.

Strategy (data-parallel over the flat pair list, per sharding hint):
  - Host: gather each pair's rows, orient every pair so sign=+1 (swap
    i/j when y_i<y_j; ties contribute 0 via wd=0), l0-shift the logits
    (softmax shift invariance), fold |dy|*w_pair into a single wd plane,
    and pack per core a (128, 16384) bf16 matrix whose partition dim
    interleaves 31 pair-groups x 4 shifted logits (+ a constant
    zero-logit row that exp turns into the softmax "+1"), with the i/j
    sides of a pair in adjacent columns.  total_weight is a pure
    function of the inputs (no softmax), summed on host.
  - Device (per core): exp on ACT; Z=1+sum(e) and W=sum(c*e) via
    128x32-column-tiled matmuls on the otherwise-idle Tensor engine;
    1/Z via the single-instruction DVE reciprocal_approx_fast;
    s=W*(1/Z) and delta=s_i-s_j on DVE; softplus(-delta)=ln(1+exp(-d))
    on ACT (exp+ln share one table set); fused multiply-by-wd +
    per-partition reduce on DVE, chained across rounds via the reduce
    initial-value operand.
  - Host: sum the 8x128 loss partials, divide by host total_weight.

Computes exactly the reference quantity; only rows referenced by pairs
contribute, so unpaired rows need not be touched.
"""

import numpy as np

import concourse.bacc as bacc
import concourse.mybir as mybir
from concourse.bass_utils import run_bass_kernel_spmd
from concourse.tile import TileContext

NCORES = 8
NPAIRS = 2_000_000
PC = NPAIRS // NCORES   # 250_000 pairs per core
P = 128

G = 31                  # pair-groups per column (partition = 4*g + c)
ONES_ROW = 124          # constant zero-logit row -> exp() == 1 (the +1 in Z)
F = 16_384              # x columns per core
PC_PAD = (F // 2) * G   # 253_952 padded pairs per core >= PC
SRC = 4_096             # x columns per (full) super-round
NJ = 2                  # PSUM sub-chunks per full super-round (2048 cols)
NK = 4                  # matmul partition-blocks per sub-chunk
MB = 512                # matmul moving free dim (one PSUM bank)
TD = MB // 2            # delta columns per (j, k) block
# Short rounds at the ends shrink pipeline fill (first exp waits on a
# 0.5MB DMA, not 1MB) and the serial drain through the 8-stage tail.
SR_COLS = [2048, 2048, 4096, 4096, 2048, 2048]
NSR = len(SR_COLS)
assert sum(SR_COLS) == F

EPS = 1e-8

f32 = mybir.dt.float32
bf16 = mybir.dt.bfloat16
AF = mybir.ActivationFunctionType
ALU = mybir.AluOpType


def _pin_act_tables(arch):
    """Make every ACT function we use first-match to one table set that
    contains both exp and ln, so the kernel needs a single
    ACT_TABLE_LOAD instead of thrashing between the exp-only and
    ln-only sets (1.3us per reload).  Only membership of the cached
    selection dict is edited; set indices (act_func_set_id) and the
    real on-device tables are untouched, so lowering stays correct.
    """
    from concourse.hw_specs import get_activation_tables

    tabs = get_activation_tables(arch)
    ours = {AF.Exp, AF.Ln}
    combined = None
    for name, fns in tabs.items():
        if ours <= fns:
            combined = name
            break
    if combined is None:
        return
    for name, fns in tabs.items():
        if name != combined:
            fns -= ours


def _build():
    nc = bacc.Bacc("TRN2", target_bir_lowering=False)
    _pin_act_tables(nc.m.arch)
    X = nc.dram_tensor("x", [P, F], bf16, kind="ExternalInput")
    WD = nc.dram_tensor("wd", [P, F // 8], bf16, kind="ExternalInput")
    WZT = nc.dram_tensor("wzt", [P, 32], bf16, kind="ExternalInput")
    WWT = nc.dram_tensor("wwt", [P, 32], bf16, kind="ExternalInput")
    OUT = nc.dram_tensor("out", [P, 1], f32, kind="ExternalOutput")

    with TileContext(nc) as tc:
        with (
            tc.tile_pool(name="io", bufs=1) as io,
            tc.tile_pool(name="ew", bufs=1) as ew,
            tc.tile_pool(name="ps", bufs=2, space="PSUM") as ps,
            tc.tile_pool(name="s1", bufs=2) as s1,
            tc.tile_pool(name="cst", bufs=1) as cst,
            tc.tile_pool(name="acc", bufs=1) as accp,
        ):
            sr_off = np.cumsum([0] + SR_COLS[:-1]).tolist()

            # Input DMAs first: the first exp waits on x0, so x wins the
            # queue; wz/ww are tiny; wd (512KB, first read by the sr0
            # reduce) goes after the first two x rounds.
            xts = []
            for sr in range(NSR):
                cols = SR_COLS[sr]
                xt = io.tile([P, cols], bf16, tag=f"x{sr}", name=f"x{sr}")
                nc.sync.dma_start(out=xt[:],
                                  in_=X[:, sr_off[sr]:sr_off[sr] + cols])
                xts.append(xt)
                if sr == 1:
                    wz = cst.tile([P, 32], bf16, tag="wz", name="wz")
                    nc.sync.dma_start(out=wz[:], in_=WZT[:, :])
                    ww = cst.tile([P, 32], bf16, tag="ww", name="ww")
                    nc.sync.dma_start(out=ww[:], in_=WWT[:, :])
                    wdt = cst.tile([P, F // 8], bf16, tag="wdt", name="wdt")
                    nc.sync.dma_start(out=wdt[:], in_=WD[:, :])

            accs = [accp.tile([P, 1], f32, tag=f"acc{i}", name=f"acc{i}")
                    for i in range(NSR)]

            def super_round(sr):
                cols = SR_COLS[sr]
                nj = cols // (NK * MB)
                xt = xts[sr]
                E = ew.tile([P, cols], bf16, tag=f"E{sr}", name=f"E{sr}")
                nc.scalar.activation(E[:], xt[:], AF.Exp)

                Zt = ps.tile([P, NJ, MB], f32, tag="Z", name=f"Z{sr}")
                Wt = ps.tile([P, NJ, MB], f32, tag="W", name=f"W{sr}")
                for j in range(nj):
                    for k in range(NK):
                        rhs = E[:, j * (NK * MB) + k * MB:
                                j * (NK * MB) + (k + 1) * MB]
                        nc.tensor.matmul(
                            Zt[32 * k:32 * (k + 1), j], wz[:, :], rhs,
                            start=True, stop=True, tile_position=(0, 32 * k))
                    for k in range(NK):
                        rhs = E[:, j * (NK * MB) + k * MB:
                                j * (NK * MB) + (k + 1) * MB]
                        nc.tensor.matmul(
                            Wt[32 * k:32 * (k + 1), j], ww[:, :], rhs,
                            start=True, stop=True, tile_position=(0, 32 * k))

                RZ = s1.tile([P, NJ, MB], f32, tag="RZ", name=f"RZ{sr}")
                nc.vector.reciprocal_approx_fast(out=RZ[:, :nj],
                                                 in_=Zt[:, :nj])
                S = s1.tile([P, NJ, TD, 2], bf16, tag="S", name=f"S{sr}")
                nc.vector.tensor_mul(out=S[:, :nj], in0=Wt[:, :nj],
                                     in1=RZ[:, :nj])
                D = s1.tile([P, NJ, TD], bf16, tag="D", name=f"D{sr}")
                nc.vector.tensor_sub(out=D[:, :nj], in0=S[:, :nj, :, 0],
                                     in1=S[:, :nj, :, 1])
                U = s1.tile([P, NJ, TD], bf16, tag="U", name=f"U{sr}")
                nc.scalar.activation(U[:, :nj], D[:, :nj], AF.Exp, scale=-1.0)
                SP = s1.tile([P, NJ, TD], bf16, tag="SP", name=f"SP{sr}")
                nc.scalar.activation(SP[:, :nj], U[:, :nj], AF.Ln, bias=1.0)
                SC = s1.tile([P, NJ, TD], bf16, tag="SC", name=f"SC{sr}")
                wcol = sr_off[sr] // 8
                wslice = wdt[:, wcol:wcol + cols // 8]
                nc.vector.scalar_tensor_tensor(
                    out=SC[:, :nj], in0=SP[:, :nj], scalar=1.0, in1=wslice,
                    op0=ALU.mult, op1=ALU.mult, accum_out=accs[sr][:])
                if sr > 0:
                    nc.vector.tensor_add(out=accs[sr][:], in0=accs[sr][:],
                                         in1=accs[sr - 1][:])

            for sr in range(NSR):
                super_round(sr)

            nc.sync.dma_start(out=OUT[:, :], in_=accs[NSR - 1][:])

    nc.compile()
    return nc


_NC_CACHE = {}


def _get_nc():
    if "nc" not in _NC_CACHE:
        _NC_CACHE["nc"] = _build()
    return _NC_CACHE["nc"]


def _weights():
    # lhsT [K=128, M=32]: column g (< G) sums the 4 class-exps of group g;
    # WZT also picks up the constant-1 row (softmax +1).  Column 31 is a
    # padding output fed by all rows so its Z/W stay wholesome (no 1/0 in
    # reciprocal); its wd is always 0 so it never contributes.
    wzt = np.zeros((P, 32), np.float32)
    wwt = np.zeros((P, 32), np.float32)
    for g in range(G):
        for c in range(4):
            wzt[4 * g + c, g] = 1.0
            wwt[4 * g + c, g] = float(c + 1)
    wzt[ONES_ROW, :G] = 1.0
    wzt[:, 31] = 1.0
    wwt[:, 31] = 1.0
    return wzt, wwt


def _prepare(inputs, targets, cluster_ids, sample_weight, pair_i, pair_j):
    import ml_dtypes

    bf = ml_dtypes.bfloat16
    x = np.ascontiguousarray(np.asarray(inputs), dtype=np.float32)
    t = np.asarray(targets)
    w = np.asarray(sample_weight, dtype=np.float32)
    pi = np.asarray(pair_i).astype(np.int64, copy=False)
    pj = np.asarray(pair_j).astype(np.int64, copy=False)

    dy = (t[pi] - t[pj]).astype(np.int64)
    swap = dy < 0
    pi2 = np.where(swap, pj, pi)
    pj2 = np.where(swap, pi, pj)
    dist = np.abs(dy).astype(np.float32)

    li = x[pi2]                         # (NPAIRS, 5), oriented so s_i-s_j
    lj = x[pj2]
    lsi = li[:, 1:5] - li[:, 0:1]       # l0-shift: softmax shift-invariant
    lsj = lj[:, 1:5] - lj[:, 0:1]

    wp = 0.5 * (w[pi] + w[pj])          # symmetric under swap
    wd = dist * wp                      # 0 exactly for ties (inactive)
    twa = float((wp * (dist != 0)).sum(dtype=np.float64))

    wzt, wwt = _weights()
    wzt = wzt.astype(bf)
    wwt = wwt.astype(bf)

    B = F // 2
    maps = []
    for kcore in range(NCORES):
        sl = slice(kcore * PC, (kcore + 1) * PC)

        lsi_p = np.zeros((PC_PAD, 4), np.float32)
        lsi_p[:PC] = lsi[sl]
        lsj_p = np.zeros((PC_PAD, 4), np.float32)
        lsj_p[:PC] = lsj[sl]
        wd_p = np.zeros(PC_PAD, np.float32)
        wd_p[:PC] = wd[sl]

        # x_dev[4g+c, 2b+side] = logit c of side of pair q = G*b+g
        lsi_r = lsi_p.reshape(B, G, 4)          # [b, g, c]
        lsj_r = lsj_p.reshape(B, G, 4)
        x4 = np.stack([lsi_r, lsj_r], axis=3)   # [b, g, c, side]
        x_dev = np.zeros((P, F), np.float32)
        x_dev[:4 * G] = x4.transpose(1, 2, 0, 3).reshape(4 * G, F)
        x_dev = np.ascontiguousarray(x_dev).astype(bf)

        # wd_dev[32k+g, u*TD+t] = wd[q], q = G*(u*4*TD + k*TD + t) + g,
        # where u indexes the 8 uniform 2048-column sub-chunks.
        NU = F // 2048
        wd_r = wd_p.reshape(NU, NK, TD, G)       # [u, k, t, g]
        wd_r = wd_r.transpose(1, 3, 0, 2)        # [k, g, u, t]
        wd_dev = np.zeros((NK, 32, NU, TD), np.float32)
        wd_dev[:, :G] = wd_r
        wd_dev = np.ascontiguousarray(
            wd_dev.reshape(P, F // 8)).astype(bf)

        maps.append({"x": x_dev, "wd": wd_dev, "wzt": wzt, "wwt": wwt})
    return maps, twa


def _run(in_maps, trace=False, **kw):
    nc = _get_nc()
    return run_bass_kernel_spmd(nc, in_maps, list(range(NCORES)), trace=trace, **kw)


def kernel(inputs, targets, cluster_ids, sample_weight, pair_i, pair_j):
    in_maps, twa = _prepare(inputs, targets, cluster_ids, sample_weight,
                            pair_i, pair_j)
    res = _run(in_maps)
    tl = 0.0
    for k in range(NCORES):
        o = res.results[k]["out"]
        tl += float(o[:, 0].sum(dtype=np.float64))
    return np.float32(tl / (twa + EPS))
